# revision 32
# baseline (speedup 1.0000x reference)
"""Multi-head causal attention (B=4, S=2048, D=1024, H=16) on 8 TRN2 NeuronCores.

Sharding: batch x head-group. Core c handles batch c//2 and heads
8*(c%2) .. 8*(c%2)+8 (tensor parallel over heads). Each core computes its
8 heads' attention plus partial output projections; the host sums the four
partials per batch and adds b_out.

Device pipeline (per core) — single woven instruction stream:
  - head pairs processed sequentially (oT needs only 2 PSUM banks), which
    frees 2 PSUM banks for a projection-chain pool that stays live through
    the attention phase: qk/v/out-projection chains are emitted as PE
    filler between attention events, paced ~RATE matmuls per event, so the
    PE never drains while ScalarE runs exp and ScalarE never waits on
    scores.
  - inputs live in per-kind mega-tiles (xTa/wqka/wva/woa) so each load
    tier is ONE DMA descriptor (the Sync engine serializes descriptor
    issue at ~600ns each; per-chunk descriptors starved the ramp).
  - scores via row-packed (tile_position) fp16 matmuls in S^T [k, q]
    layout, exp on ScalarE straight out of PSUM, fp16 P with a single
    shared 128x128 triangle mask tile for diagonal blocks (all causal
    diagonal blocks share one shifted pattern); fully-masked blocks
    skipped, fully-masked leading columns excluded from score matmul, exp,
    AND attn@V (N-trimmed).
  - attn@V with lhsT = [v_h | ones] (M=65): row 64 accumulates softmax
    denominators; normalization = reciprocal_approx_fast + GpSimd
    partition broadcast + multiply.
  - out-projection split into two f16 partials (pairs 0-1 / pairs 2-3) so
    d-chains weave in as soon as their pair-group's rows normalize; host
    sums partials.
"""
import collections

import numpy as np

import concourse.bass as bass
import concourse.tile as tile
from concourse import bacc, mybir
from concourse import bass_utils

B, S, D, H, HD = 4, 2048, 1024, 16, 64
NCORES = 8
HPC = H // 2          # heads per core (8)
NPAIR = HPC // 2      # head pairs per core (4)
DC = HPC * HD         # attn dims per core (512)
QT = 512              # q tile (free dim of S^T)
KT = 128              # k tile (partition dim of S^T)
NQT = S // QT         # 4
NKT = S // KT         # 16
NTT = S // 128        # 16 token tiles
NCH = D // 128        # 8 d_model chunks
SCALE = HD ** -0.5
LAG = 4               # events of exp lookahead before attn@V
RATE = 3.0               # filler matmul-slots released per attention event


F32 = mybir.dt.float32
F16 = mybir.dt.float16

_cache = {}


def _classify_mask(mask):
    """Per (kt, qt) block: 0=skip (all masked), 1=full (none masked), 2=partial.

    Partial blocks must match a single shared shifted triangle:
    keep[k, mlo + q'] == (q' >= k). True for any causal (triu k=1) mask.
    """
    mask = np.asarray(mask).astype(bool)
    classes = np.zeros((NKT, NQT), np.int8)
    bounds = {}
    tri_ref = (np.arange(QT)[None, :] >= np.arange(KT)[:, None])  # [128, 512]
    for qt in range(NQT):
        mb = mask[qt * QT:(qt + 1) * QT, :]          # [512, S] (q, k)
        for kt in range(NKT):
            blk = mb[:, kt * KT:(kt + 1) * KT]       # [512, 128] (q, k)
            if blk.all():
                classes[kt, qt] = 0
            elif not blk.any():
                classes[kt, qt] = 1
            else:
                classes[kt, qt] = 2
                tilev = (~blk).T.astype(np.float32)  # [128, 512] (k, q), 1=keep
                col_masked = tilev.min(axis=0) == 0.0
                col_dead = tilev.max(axis=0) == 0.0
                zlo = 0
                while zlo < QT and col_dead[zlo]:
                    zlo += 1
                nz = np.nonzero(col_masked[zlo:])[0]
                if len(nz):
                    mlo, mhi = zlo + int(nz[0]), zlo + int(nz[-1]) + 1
                else:
                    mlo, mhi = zlo, zlo
                bounds[(kt, qt)] = (zlo, mlo, mhi)
                w = mhi - mlo
                if not np.array_equal(tilev[:, mlo:mhi],
                                      tri_ref[:, :w].astype(np.float32)):
                    raise NotImplementedError(
                        "mask partial block is not the shared causal triangle")
    return classes, bounds


def _build(classes, bounds):
    nc = bacc.Bacc("TRN2", target_bir_lowering=False, debug=False,
                   num_devices=NCORES)

    # All inputs host-swizzled to [128 partitions, contiguous payload] so
    # every DMA moves maximal contiguous lines (packet count, not bytes,
    # dominates DMA queue time).
    xT_d = nc.dram_tensor("xT", [128, NQT * NCH * QT], F16,
                          kind="ExternalInput")
    wqk_d = nc.dram_tensor("wqk", [128, 8 * NCH * 128], F16,
                           kind="ExternalInput")
    wv_d = nc.dram_tensor("wv", [128, NCH * DC], F16, kind="ExternalInput")
    wo_d = nc.dram_tensor("wo", [128, NPAIR * D], F16, kind="ExternalInput")
    bqk_d = nc.dram_tensor("bqk", [128, 2 * NPAIR], F32, kind="ExternalInput")
    bv_d = nc.dram_tensor("bv", [1, DC], F32, kind="ExternalInput")
    tri_d = nc.dram_tensor("tri", [KT, KT], F16, kind="ExternalInput")
    outA_d = nc.dram_tensor("outA", [S, D], F16, kind="ExternalOutput")

    with tile.TileContext(nc) as tc:
        with (
            tc.tile_pool(name="persist", bufs=1) as persist,
            tc.tile_pool(name="bigpool", bufs=1) as bigpool,
            tc.tile_pool(name="ppool", bufs=8) as ppool,
            tc.tile_pool(name="spool", bufs=4) as spool,
            tc.tile_pool(name="dcopy", bufs=8) as dcopy,
            tc.tile_pool(name="psS", bufs=2, space="PSUM") as psS,
            tc.tile_pool(name="psO", bufs=1, space="PSUM") as psO,
            tc.tile_pool(name="psA", bufs=2, space="PSUM") as psA,
        ):
            # ---- persistent tiles -------------------------------------
            qkT = [bigpool.tile([128, S], F16, name=f"qkT{p}", tag="qk",
                                bufs=8) for p in range(2 * NPAIR)]
            vext = [persist.tile([128, HPC, HD + 1], F16, name=f"vx{t}",
                                 tag=f"vx{t}") for t in range(NTT)]
            tri = persist.tile([KT, KT], F16, name="tri")
            outTn = [bigpool.tile([128, S], F16, name=f"oTn{p}", tag="big",
                                  bufs=4) for p in range(NPAIR)]
            bqk_sb = persist.tile([128, 2 * NPAIR], F32)
            bv_bc = persist.tile([128, DC], F32)
            # layouts: xTa[p, qt, ch, col], wqka[p, slice, ch, col]
            xTa = persist.tile([128, NQT, NCH, QT], F16, name="xTa",
                               tag="xTa")
            wqka = persist.tile([128, 8, NCH, 128], F16, name="wqka",
                                tag="wqka")
            wva = persist.tile([128, NCH, DC], F16, name="wva", tag="wva")
            woa = persist.tile([128, NPAIR, D], F16, name="woa", tag="woa")

            xT_s = xT_d.ap().rearrange("p (t c n) -> p t c n", c=NCH, n=QT)
            wqk_s = wqk_d.ap().rearrange("p (s c n) -> p s c n", c=NCH, n=128)

            # ---- ramp-critical DMAs: descriptors issue in PARALLEL from
            # several engines (each DMA_DIRECT2D costs 0.6-2.6us of issue
            # time on its engine's queue; serial issue starved the ramp).
            # DMA engines drain descriptors roughly FIFO by arrival, so the
            # first qk chain's weights+x lead on the scalar queue.
            nc.scalar.dma_start(wqka[:, 0], wqk_s[:, 0])
            nc.scalar.dma_start(xTa[:, 0, 0:4], xT_s[:, 0, 0:4])
            nc.scalar.dma_start(xTa[:, 0, 4:8], xT_s[:, 0, 4:8])
            nc.sync.dma_start(bqk_sb, bqk_d.ap())
            nc.sync.dma_start(tri, tri_d.ap())
            nc.sync.dma_start(wqka[:, 4], wqk_s[:, 4])
            nc.sync.dma_start(
                bv_bc,
                bass.AP(tensor=bv_d, offset=0, ap=[[0, 128], [1, DC]]))
            nc.gpsimd.dma_start(wva, wv_d.ap().rearrange(
                "p (c n) -> p c n", n=DC))

            guard = persist.tile([1, 64], F16, name="guard")
            guard_n = [0]

            def deferred_dma(trig, dst_probe, dst, src):
                """Emit DMA gated behind trig via a WAR-creating dummy read."""
                g = guard_n[0]
                guard_n[0] += 1
                nc.vector.tensor_add(guard[0:1, g:g + 1], trig, dst_probe)
                nc.sync.dma_start(dst, src)

            def emit_tier(tier):
                # tier 2: x tt1-2 + wqk q1/k1 once the ramp's first chain
                # runs; tier 3: x tt3 + wqk q23/k23; tier 4: w_out.
                if tier == 2:
                    trig = qkT[0][0:1, 0:1]
                    deferred_dma(trig, xTa[0:1, 1:2, 0:1, 0:1],
                                 xTa[:, 1:3], xT_s[:, 1:3])
                    deferred_dma(trig, wqka[0:1, 1:2, 0:1, 0:1],
                                 wqka[:, 1], wqk_s[:, 1])
                    deferred_dma(trig, wqka[0:1, 5:6, 0:1, 0:1],
                                 wqka[:, 5], wqk_s[:, 5])
                elif tier == 3:
                    trig = qkT[0][0:1, QT:QT + 1]
                    deferred_dma(trig, xTa[0:1, 3:4, 0:1, 0:1],
                                 xTa[:, 3], xT_s[:, 3])
                    deferred_dma(trig, wqka[0:1, 2:3, 0:1, 0:1],
                                 wqka[:, 2:4], wqk_s[:, 2:4])
                    deferred_dma(trig, wqka[0:1, 6:7, 0:1, 0:1],
                                 wqka[:, 6:8], wqk_s[:, 6:8])
                else:
                    trig = qkT[1][0:1, 0:1]
                    deferred_dma(trig, woa[0:1, 0:1, 0:1], woa,
                                 wo_d.ap().rearrange("p (r n) -> p r n", n=D))

            # ---- chain emitters ---------------------------------------
            def emit_qk_chain(p, tt):
                """qkT[p][:, tt*QT:...] = (w_qk[:, p-slice].T @ x.T) + bias."""
                ps = psA.tile([128, QT], F32, tag="pa", name=f"psqk{p}_{tt}")
                for ch in range(NCH):
                    nc.tensor.matmul(
                        ps, wqka[:, p, ch, :], xTa[:, tt, ch, :],
                        start=(ch == 0), stop=(ch == NCH - 1))
                nc.vector.tensor_scalar_add(
                    qkT[p][:, tt * QT:(tt + 1) * QT], ps, bqk_sb[:, p:p + 1])
                if (p, tt) == (0, 1):
                    emit_tier(3)
                elif (p, tt) == (1, 0):
                    emit_tier(4)

            def emit_v_chain(tt):
                """vext[tt] <- x[tt-tokens] @ w_v + bias, plus ones column."""
                ps = psA.tile([128, DC], F32, tag="pa", name=f"psv{tt}")
                t4, r4 = tt // 4, (tt % 4) * 128
                for ch in range(NCH):
                    nc.tensor.matmul(
                        ps, xTa[:, t4, ch, r4:r4 + 128], wva[:, ch, :],
                        start=(ch == 0), stop=(ch == NCH - 1))
                src3 = ps.rearrange("p (h d) -> p h d", h=HPC)
                bv3 = bv_bc.rearrange("p (h d) -> p h d", h=HPC)
                nc.vector.tensor_add(vext[tt][:, :, 0:HD], src3, bv3)
                nc.vector.memset(vext[tt][:, :, HD:HD + 1], 1.0)

            def emit_d_chain(tt, nt):
                """Out-projection token block tt, cols nt*QT: all 4 pairs."""
                pso = psA.tile([128, QT], F32, name=f"pso{tt}_{nt}",
                               tag="pa")
                for p in range(NPAIR):
                    nc.tensor.matmul(
                        pso, outTn[p][:, tt * 128:(tt + 1) * 128],
                        woa[:, p, nt * QT:(nt + 1) * QT],
                        start=(p == 0), stop=(p == NPAIR - 1))
                ot = dcopy.tile([128, QT], F16, tag="oc")
                nc.vector.tensor_copy(ot, pso)
                nc.sync.dma_start(
                    outA_d.ap()[tt * 128:(tt + 1) * 128,
                                nt * QT:(nt + 1) * QT], ot)

            # ---- filler machinery -------------------------------------
            emitted = set()
            filler_q = collections.deque()
            state = {"quota": 0.0}

            def emit_unit(key):
                if key in emitted:
                    return
                emitted.add(key)
                if key[0] == "qk":
                    emit_qk_chain(key[1], key[2])
                elif key[0] == "v":
                    emit_v_chain(key[1])
                else:
                    emit_d_chain(key[1], key[2])

            def pop_fillers():
                while filler_q:
                    cost, key = filler_q[0]
                    if key in emitted:
                        filler_q.popleft()
                        continue
                    if state["quota"] < cost:
                        break
                    filler_q.popleft()
                    state["quota"] -= cost
                    emit_unit(key)

            # static queue: p0's remaining qk/v deps, then later pairs' qk
            for tt in range(1, NQT):
                filler_q.append((8, ("qk", 0, tt)))
                filler_q.append((8, ("qk", NPAIR, tt)))
                for kt in range(4 * tt, 4 * tt + 4):
                    filler_q.append((8, ("v", kt)))
            for p in range(1, NPAIR):
                for tt in range(NQT):
                    filler_q.append((8, ("qk", p, tt)))
                    filler_q.append((8, ("qk", NPAIR + p, tt)))

            # ---- attention emitters -----------------------------------
            oT_live = {}
            pAB_live = {}

            def emit_sexp(ev):
                p, qt, kt, first, last = ev
                qTp, kTp = qkT[p], qkT[NPAIR + p]
                if first:
                    oT_live[(p, qt)] = [
                        psO.tile([HD + 1, QT], F32,
                                 name=f"o{p}_{qt}_{h}", tag=f"o_{h}")
                        for h in range(2)]
                zlo, mlo, mhi = (0, 0, 0) if classes[kt, qt] == 1 \
                    else bounds[(kt, qt)]
                sAB = psS.tile([128, 2, QT], F32, tag="sAB",
                               name=f"s{p}_{qt}_{kt}")
                for h in range(2):
                    nc.tensor.matmul(
                        sAB[:, h, zlo:QT],
                        kTp[64 * h:64 * h + 64, kt * KT:(kt + 1) * KT],
                        qTp[64 * h:64 * h + 64, qt * QT + zlo:(qt + 1) * QT],
                        tile_position=(64 * h, 0))
                pAB = ppool.tile([128, 2, QT], F16, tag="pAB",
                                 name=f"p{p}_{qt}_{kt}")
                nc.scalar.activation(
                    pAB[:, :, zlo:QT], sAB[:, :, zlo:QT],
                    mybir.ActivationFunctionType.Exp, scale=SCALE)
                if mhi > mlo:
                    pap = tri[:, 0:mhi - mlo]
                    bap = bass.AP(tensor=pap.tensor, offset=pap.offset,
                                  ap=[pap.ap[0], [0, 2]] + pap.ap[1:])
                    nc.vector.tensor_mul(
                        pAB[:, :, mlo:mhi], pAB[:, :, mlo:mhi], bap)
                if first and zlo:
                    # general-mask guard: first event must initialize the
                    # full oT width, so zero the dead columns and run av
                    # untrimmed (never hit for a causal mask: zlo==0).
                    nc.vector.memset(pAB[:, :, 0:zlo], 0.0)
                    zlo = 0
                pAB_live[(p, qt, kt)] = (pAB, zlo)

            def emit_av(ev):
                p, qt, kt, first, last = ev
                oT = oT_live[(p, qt)]
                pAB, zlo = pAB_live.pop((p, qt, kt))
                for h in range(2):
                    nc.tensor.matmul(
                        oT[h][:, zlo:QT], vext[kt][:, 2 * p + h, :],
                        pAB[:, h, zlo:QT], start=first, stop=last)
                if last:
                    for h in range(2):
                        den = spool.tile([1, QT], F32, tag="den",
                                         name=f"d{p}_{qt}_{h}")
                        nc.vector.tensor_copy(den, oT[h][HD:HD + 1, :])
                        rec = spool.tile([1, QT], F32, tag="rec",
                                         name=f"r{p}_{qt}_{h}")
                        nc.vector.reciprocal_approx_fast(out=rec, in_=den)
                        bc = spool.tile([HD, QT], F32, tag="bc",
                                        name=f"b{p}_{qt}_{h}")
                        nc.gpsimd.partition_broadcast(bc, rec[0:1, :])
                        nc.vector.tensor_mul(
                            outTn[p][64 * h:64 * h + 64,
                                     qt * QT:(qt + 1) * QT],
                            oT[h][0:HD, :], bc)
                    del oT_live[(p, qt)]
                    if p == 3:
                        # all pairs' rows for this qt are final;
                        # out-projection chains become emittable
                        for tt in range(4 * qt, 4 * qt + 4):
                            for nt in range(2):
                                filler_q.append((4, ("d", tt, nt)))

            all_events = []
            for p in range(NPAIR):
                for qt in range(NQT):
                    kts = [kt for kt in range(NKT) if classes[kt, qt] != 0]
                    for i, kt in enumerate(kts):
                        all_events.append(
                            (p, qt, kt, i == 0, i == len(kts) - 1))

            # ---- ramp: first qk chains, first scores, then v ----------
            # (avs need vext only LAG events in, so exp starts before the
            # v chains instead of behind them)
            with nc.named_scope("ramp"):
                emit_unit(("qk", 0, 0))
                emit_unit(("qk", NPAIR, 0))
                emit_tier(2)
                for ev in all_events[0:LAG]:
                    emit_sexp(ev)
                for kt in range(4):
                    emit_unit(("v", kt))

            # ---- main woven stream ------------------------------------
            with nc.named_scope("attn"):
                # events processed in pairs: both score pairs back-to-back,
                # then both trailing avs — halves the LDWEIGHTS stalls paid
                # entering/leaving the row-packed score matmuls.
                nev = len(all_events)
                for base in range(LAG, nev, 2):
                    evs = all_events[base:base + 2]
                    for ev in evs:
                        p, qt, kt, first, last = ev
                        emit_unit(("qk", p, qt))
                        emit_unit(("qk", NPAIR + p, kt // 4))
                        emit_unit(("v", kt))
                    for ev in evs:
                        emit_sexp(ev)
                    state["quota"] += RATE * len(evs)
                    for idx in range(base, base + len(evs)):
                        j = idx - LAG
                        if j >= 0:
                            emit_av(all_events[j])
                    pop_fillers()
                for j in range(max(0, nev - LAG), nev):
                    emit_av(all_events[j])

            with nc.named_scope("tail"):
                state["quota"] = float("inf")
                pop_fillers()

    nc.compile()
    return nc


def _prepare_inputs(x, mask, w_qkv, b_qkv, w_out):
    classes, bounds = _classify_mask(np.asarray(mask))
    tri_np = (np.arange(QT)[None, :] >= np.arange(KT)[:, None])
    tri_np = tri_np[:, :KT].astype(np.float16)      # [128, 128] triangle
    in_maps = []
    for c in range(NCORES):
        b, g = c // 2, c % 2
        h0 = g * HPC
        # all layouts swizzled to [128 partitions, contiguous payload]
        xT = (x[b].T.astype(np.float16).reshape(NCH, 128, NQT, QT)
              .transpose(1, 2, 0, 3).reshape(128, -1))
        wq = w_qkv[:, h0 * HD:h0 * HD + DC]
        wk = w_qkv[:, D + h0 * HD:D + h0 * HD + DC]
        wv = w_qkv[:, 2 * D + h0 * HD:2 * D + h0 * HD + DC]
        bq = b_qkv[h0 * HD:h0 * HD + DC]
        bk = b_qkv[D + h0 * HD:D + h0 * HD + DC]
        bv = b_qkv[2 * D + h0 * HD:2 * D + h0 * HD + DC]
        wo = w_out[h0 * HD:h0 * HD + DC, :]
        wqk = (np.concatenate([wq, wk], axis=1).astype(np.float16)
               .reshape(NCH, 128, 8, 128).transpose(1, 2, 0, 3)
               .reshape(128, -1))
        in_maps.append({
            "xT": np.ascontiguousarray(xT),
            "wqk": np.ascontiguousarray(wqk),
            "wv": np.ascontiguousarray(
                wv.astype(np.float16).reshape(NCH, 128, DC)
                .transpose(1, 0, 2).reshape(128, -1)),
            "wo": np.ascontiguousarray(
                wo.astype(np.float16).reshape(NPAIR, 128, D)
                .transpose(1, 0, 2).reshape(128, -1)),
            "bqk": np.ascontiguousarray(
                np.concatenate([bq, bk]).reshape(2 * NPAIR, 128).T
                .astype(np.float32)),
            "bv": np.ascontiguousarray(bv[None, :].astype(np.float32)),
            "tri": tri_np,
        })
    return classes, bounds, in_maps


def run(x, mask, w_qkv, b_qkv, w_out, b_out, trace=False):
    classes, bounds, in_maps = _prepare_inputs(
        x, mask, w_qkv, b_qkv, w_out)
    key = (classes.tobytes(), tuple(sorted(bounds.items())))
    if key not in _cache:
        _cache[key] = _build(classes, bounds)
    nc = _cache[key]
    res = bass_utils.run_bass_kernel_spmd(
        nc, in_maps, core_ids=list(range(NCORES)), trace=trace)
    out = np.empty((B, S, D), np.float32)
    bo = np.asarray(b_out, np.float32)
    for b in range(B):
        out[b] = (res.results[2 * b]["outA"].astype(np.float32)
                  + res.results[2 * b + 1]["outA"].astype(np.float32) + bo)
    return out, res


def kernel(x, mask, w_qkv, b_qkv, w_out, b_out):
    out, _ = run(x, mask, w_qkv, b_qkv, w_out, b_out, trace=False)
    return out


# revision 33
# speedup vs baseline: 1.0361x; 1.0361x over previous
"""Multi-head causal attention (B=4, S=2048, D=1024, H=16) on 8 TRN2 NeuronCores.

Sharding: batch x head-group. Core c handles batch c//2 and heads
8*(c%2) .. 8*(c%2)+8 (tensor parallel over heads). Each core computes its
8 heads' attention plus partial output projections; the host sums the four
partials per batch and adds b_out.

Device pipeline (per core) — single woven instruction stream:
  - head pairs processed sequentially (oT needs only 2 PSUM banks), which
    frees 2 PSUM banks for a projection-chain pool that stays live through
    the attention phase: qk/v/out-projection chains are emitted as PE
    filler between attention events, paced ~RATE matmuls per event, so the
    PE never drains while ScalarE runs exp and ScalarE never waits on
    scores.
  - inputs live in per-kind mega-tiles (xTa/wqka/wva/woa) so each load
    tier is ONE DMA descriptor (the Sync engine serializes descriptor
    issue at ~600ns each; per-chunk descriptors starved the ramp).
  - scores via row-packed (tile_position) fp16 matmuls in S^T [k, q]
    layout, exp on ScalarE straight out of PSUM, fp16 P with a single
    shared 128x128 triangle mask tile for diagonal blocks (all causal
    diagonal blocks share one shifted pattern); fully-masked blocks
    skipped, fully-masked leading columns excluded from score matmul, exp,
    AND attn@V (N-trimmed).
  - attn@V with lhsT = [v_h | ones] (M=65): row 64 accumulates softmax
    denominators; normalization = reciprocal_approx_fast + GpSimd
    partition broadcast + multiply.
  - out-projection split into two f16 partials (pairs 0-1 / pairs 2-3) so
    d-chains weave in as soon as their pair-group's rows normalize; host
    sums partials.
"""
import collections

import numpy as np

import concourse.bass as bass
import concourse.tile as tile
from concourse import bacc, mybir
from concourse import bass_utils

B, S, D, H, HD = 4, 2048, 1024, 16, 64
NCORES = 8
HPC = H // 2          # heads per core (8)
NPAIR = HPC // 2      # head pairs per core (4)
DC = HPC * HD         # attn dims per core (512)
QT = 512              # q tile (free dim of S^T)
KT = 128              # k tile (partition dim of S^T)
NQT = S // QT         # 4
NKT = S // KT         # 16
NTT = S // 128        # 16 token tiles
NCH = D // 128        # 8 d_model chunks
SCALE = HD ** -0.5
LAG = 4               # events of exp lookahead before attn@V
RATE = 2.3               # filler matmul-slots released per attention event


F32 = mybir.dt.float32
F16 = mybir.dt.float16

_cache = {}


def _classify_mask(mask):
    """Per (kt, qt) block: 0=skip (all masked), 1=full (none masked), 2=partial.

    Partial blocks must match a single shared shifted triangle:
    keep[k, mlo + q'] == (q' >= k). True for any causal (triu k=1) mask.
    """
    mask = np.asarray(mask).astype(bool)
    classes = np.zeros((NKT, NQT), np.int8)
    bounds = {}
    tri_ref = (np.arange(QT)[None, :] >= np.arange(KT)[:, None])  # [128, 512]
    for qt in range(NQT):
        mb = mask[qt * QT:(qt + 1) * QT, :]          # [512, S] (q, k)
        for kt in range(NKT):
            blk = mb[:, kt * KT:(kt + 1) * KT]       # [512, 128] (q, k)
            if blk.all():
                classes[kt, qt] = 0
            elif not blk.any():
                classes[kt, qt] = 1
            else:
                classes[kt, qt] = 2
                tilev = (~blk).T.astype(np.float32)  # [128, 512] (k, q), 1=keep
                col_masked = tilev.min(axis=0) == 0.0
                col_dead = tilev.max(axis=0) == 0.0
                zlo = 0
                while zlo < QT and col_dead[zlo]:
                    zlo += 1
                nz = np.nonzero(col_masked[zlo:])[0]
                if len(nz):
                    mlo, mhi = zlo + int(nz[0]), zlo + int(nz[-1]) + 1
                else:
                    mlo, mhi = zlo, zlo
                bounds[(kt, qt)] = (zlo, mlo, mhi)
                w = mhi - mlo
                if not np.array_equal(tilev[:, mlo:mhi],
                                      tri_ref[:, :w].astype(np.float32)):
                    raise NotImplementedError(
                        "mask partial block is not the shared causal triangle")
    return classes, bounds


def _build(classes, bounds):
    nc = bacc.Bacc("TRN2", target_bir_lowering=False, debug=False,
                   num_devices=NCORES)

    # All inputs host-swizzled to [128 partitions, contiguous payload] so
    # every DMA moves maximal contiguous lines (packet count, not bytes,
    # dominates DMA queue time).
    xT_d = nc.dram_tensor("xT", [128, NQT * NCH * QT], F16,
                          kind="ExternalInput")
    wqk_d = nc.dram_tensor("wqk", [128, 8 * NCH * 128], F16,
                           kind="ExternalInput")
    wv_d = nc.dram_tensor("wv", [128, NCH * DC], F16, kind="ExternalInput")
    wo_d = nc.dram_tensor("wo", [128, NPAIR * D], F16, kind="ExternalInput")
    bqk_d = nc.dram_tensor("bqk", [128, 2 * NPAIR], F32, kind="ExternalInput")
    bv_d = nc.dram_tensor("bv", [1, DC], F32, kind="ExternalInput")
    tri_d = nc.dram_tensor("tri", [KT, KT], F16, kind="ExternalInput")
    outA_d = nc.dram_tensor("outA", [S, D], F16, kind="ExternalOutput")

    with tile.TileContext(nc) as tc:
        with (
            tc.tile_pool(name="persist", bufs=1) as persist,
            tc.tile_pool(name="bigpool", bufs=1) as bigpool,
            tc.tile_pool(name="ppool", bufs=8) as ppool,
            tc.tile_pool(name="spool", bufs=4) as spool,
            tc.tile_pool(name="dcopy", bufs=8) as dcopy,
            tc.tile_pool(name="psS", bufs=2, space="PSUM") as psS,
            tc.tile_pool(name="psO", bufs=1, space="PSUM") as psO,
            tc.tile_pool(name="psA", bufs=2, space="PSUM") as psA,
        ):
            # ---- persistent tiles -------------------------------------
            qkT = [bigpool.tile([128, S], F16, name=f"qkT{p}", tag="qk",
                                bufs=8) for p in range(2 * NPAIR)]
            vext = [persist.tile([128, HPC, HD + 1], F16, name=f"vx{t}",
                                 tag=f"vx{t}") for t in range(NTT)]
            tri = persist.tile([KT, KT], F16, name="tri")
            outTn = [bigpool.tile([128, S], F16, name=f"oTn{p}", tag="big",
                                  bufs=4) for p in range(NPAIR)]
            bqk_sb = persist.tile([128, 2 * NPAIR], F32)
            bv_bc = persist.tile([128, DC], F32)
            # layouts: xTa[p, qt, ch, col], wqka[p, slice, ch, col]
            xTa = persist.tile([128, NQT, NCH, QT], F16, name="xTa",
                               tag="xTa")
            wqka = persist.tile([128, 8, NCH, 128], F16, name="wqka",
                                tag="wqka")
            wva = persist.tile([128, NCH, DC], F16, name="wva", tag="wva")
            woa = persist.tile([128, NPAIR, D], F16, name="woa", tag="woa")

            xT_s = xT_d.ap().rearrange("p (t c n) -> p t c n", c=NCH, n=QT)
            wqk_s = wqk_d.ap().rearrange("p (s c n) -> p s c n", c=NCH, n=128)

            # ---- ramp-critical DMAs: descriptors issue in PARALLEL from
            # several engines (each DMA_DIRECT2D costs 0.6-2.6us of issue
            # time on its engine's queue; serial issue starved the ramp).
            # DMA engines drain descriptors roughly FIFO by arrival, so the
            # first qk chain's weights+x lead on the scalar queue.
            nc.scalar.dma_start(wqka[:, 0], wqk_s[:, 0])
            nc.scalar.dma_start(xTa[:, 0, 0:4], xT_s[:, 0, 0:4])
            nc.scalar.dma_start(xTa[:, 0, 4:8], xT_s[:, 0, 4:8])
            nc.sync.dma_start(bqk_sb, bqk_d.ap())
            nc.sync.dma_start(tri, tri_d.ap())
            nc.sync.dma_start(wqka[:, 4], wqk_s[:, 4])
            nc.sync.dma_start(
                bv_bc,
                bass.AP(tensor=bv_d, offset=0, ap=[[0, 128], [1, DC]]))
            nc.gpsimd.dma_start(wva, wv_d.ap().rearrange(
                "p (c n) -> p c n", n=DC))

            guard = persist.tile([1, 64], F16, name="guard")
            guard_n = [0]

            def deferred_dma(trig, dst_probe, dst, src):
                """Emit DMA gated behind trig via a WAR-creating dummy read."""
                g = guard_n[0]
                guard_n[0] += 1
                nc.vector.tensor_add(guard[0:1, g:g + 1], trig, dst_probe)
                nc.sync.dma_start(dst, src)

            def emit_tier(tier):
                # tier 2: x tt1-2 + wqk q1/k1 once the ramp's first chain
                # runs; tier 3: x tt3 + wqk q23/k23; tier 4: w_out.
                if tier == 2:
                    trig = qkT[0][0:1, 0:1]
                    deferred_dma(trig, xTa[0:1, 1:2, 0:1, 0:1],
                                 xTa[:, 1:3], xT_s[:, 1:3])
                    deferred_dma(trig, wqka[0:1, 1:2, 0:1, 0:1],
                                 wqka[:, 1], wqk_s[:, 1])
                    deferred_dma(trig, wqka[0:1, 5:6, 0:1, 0:1],
                                 wqka[:, 5], wqk_s[:, 5])
                elif tier == 3:
                    trig = qkT[0][0:1, QT:QT + 1]
                    deferred_dma(trig, xTa[0:1, 3:4, 0:1, 0:1],
                                 xTa[:, 3], xT_s[:, 3])
                    deferred_dma(trig, wqka[0:1, 2:3, 0:1, 0:1],
                                 wqka[:, 2:4], wqk_s[:, 2:4])
                    deferred_dma(trig, wqka[0:1, 6:7, 0:1, 0:1],
                                 wqka[:, 6:8], wqk_s[:, 6:8])
                else:
                    trig = qkT[1][0:1, 0:1]
                    deferred_dma(trig, woa[0:1, 0:1, 0:1], woa,
                                 wo_d.ap().rearrange("p (r n) -> p r n", n=D))

            # ---- chain emitters ---------------------------------------
            def emit_qk_chain(p, tt):
                """qkT[p][:, tt*QT:...] = (w_qk[:, p-slice].T @ x.T) + bias."""
                ps = psA.tile([128, QT], F32, tag="pa", name=f"psqk{p}_{tt}")
                for ch in range(NCH):
                    nc.tensor.matmul(
                        ps, wqka[:, p, ch, :], xTa[:, tt, ch, :],
                        start=(ch == 0), stop=(ch == NCH - 1))
                nc.vector.tensor_scalar_add(
                    qkT[p][:, tt * QT:(tt + 1) * QT], ps, bqk_sb[:, p:p + 1])
                if (p, tt) == (0, 1):
                    emit_tier(3)
                elif (p, tt) == (1, 0):
                    emit_tier(4)

            def emit_v_chain(tt):
                """vext[tt] <- x[tt-tokens] @ w_v + bias, plus ones column."""
                ps = psA.tile([128, DC], F32, tag="pa", name=f"psv{tt}")
                t4, r4 = tt // 4, (tt % 4) * 128
                for ch in range(NCH):
                    nc.tensor.matmul(
                        ps, xTa[:, t4, ch, r4:r4 + 128], wva[:, ch, :],
                        start=(ch == 0), stop=(ch == NCH - 1))
                src3 = ps.rearrange("p (h d) -> p h d", h=HPC)
                bv3 = bv_bc.rearrange("p (h d) -> p h d", h=HPC)
                nc.vector.tensor_add(vext[tt][:, :, 0:HD], src3, bv3)
                nc.vector.memset(vext[tt][:, :, HD:HD + 1], 1.0)

            def emit_d_chain(tt, nt):
                """Out-projection token block tt, cols nt*QT: all 4 pairs."""
                pso = psA.tile([128, QT], F32, name=f"pso{tt}_{nt}",
                               tag="pa")
                for p in range(NPAIR):
                    nc.tensor.matmul(
                        pso, outTn[p][:, tt * 128:(tt + 1) * 128],
                        woa[:, p, nt * QT:(nt + 1) * QT],
                        start=(p == 0), stop=(p == NPAIR - 1))
                ot = dcopy.tile([128, QT], F16, tag="oc")
                nc.vector.tensor_copy(ot, pso)
                nc.sync.dma_start(
                    outA_d.ap()[tt * 128:(tt + 1) * 128,
                                nt * QT:(nt + 1) * QT], ot)

            # ---- filler machinery -------------------------------------
            emitted = set()
            filler_q = collections.deque()
            state = {"quota": 0.0}

            def emit_unit(key):
                if key in emitted:
                    return
                emitted.add(key)
                if key[0] == "qk":
                    emit_qk_chain(key[1], key[2])
                elif key[0] == "v":
                    emit_v_chain(key[1])
                else:
                    emit_d_chain(key[1], key[2])

            def pop_fillers():
                while filler_q:
                    cost, key = filler_q[0]
                    if key in emitted:
                        filler_q.popleft()
                        continue
                    if state["quota"] < cost:
                        break
                    filler_q.popleft()
                    state["quota"] -= cost
                    emit_unit(key)

            # static queue: p0's remaining qk/v deps, then later pairs' qk
            for tt in range(1, NQT):
                filler_q.append((8, ("qk", 0, tt)))
                filler_q.append((8, ("qk", NPAIR, tt)))
                for kt in range(4 * tt, 4 * tt + 4):
                    filler_q.append((8, ("v", kt)))
            for p in range(1, NPAIR):
                for tt in range(NQT):
                    filler_q.append((8, ("qk", p, tt)))
                    filler_q.append((8, ("qk", NPAIR + p, tt)))

            # ---- attention emitters -----------------------------------
            oT_live = {}
            pAB_live = {}

            def emit_sexp(ev):
                p, qt, kt, first, last = ev
                qTp, kTp = qkT[p], qkT[NPAIR + p]
                if first:
                    oT_live[(p, qt)] = [
                        psO.tile([HD + 1, QT], F32,
                                 name=f"o{p}_{qt}_{h}", tag=f"o_{h}")
                        for h in range(2)]
                zlo, mlo, mhi = (0, 0, 0) if classes[kt, qt] == 1 \
                    else bounds[(kt, qt)]
                sAB = psS.tile([128, 2, QT], F32, tag="sAB",
                               name=f"s{p}_{qt}_{kt}")
                for h in range(2):
                    nc.tensor.matmul(
                        sAB[:, h, zlo:QT],
                        kTp[64 * h:64 * h + 64, kt * KT:(kt + 1) * KT],
                        qTp[64 * h:64 * h + 64, qt * QT + zlo:(qt + 1) * QT],
                        tile_position=(64 * h, 0))
                pAB = ppool.tile([128, 2, QT], F16, tag="pAB",
                                 name=f"p{p}_{qt}_{kt}")
                nc.scalar.activation(
                    pAB[:, :, zlo:QT], sAB[:, :, zlo:QT],
                    mybir.ActivationFunctionType.Exp, scale=SCALE)
                if mhi > mlo:
                    pap = tri[:, 0:mhi - mlo]
                    bap = bass.AP(tensor=pap.tensor, offset=pap.offset,
                                  ap=[pap.ap[0], [0, 2]] + pap.ap[1:])
                    nc.vector.tensor_mul(
                        pAB[:, :, mlo:mhi], pAB[:, :, mlo:mhi], bap)
                if first and zlo:
                    # general-mask guard: first event must initialize the
                    # full oT width, so zero the dead columns and run av
                    # untrimmed (never hit for a causal mask: zlo==0).
                    nc.vector.memset(pAB[:, :, 0:zlo], 0.0)
                    zlo = 0
                pAB_live[(p, qt, kt)] = (pAB, zlo)

            def emit_av(ev):
                p, qt, kt, first, last = ev
                oT = oT_live[(p, qt)]
                pAB, zlo = pAB_live.pop((p, qt, kt))
                for h in range(2):
                    nc.tensor.matmul(
                        oT[h][:, zlo:QT], vext[kt][:, 2 * p + h, :],
                        pAB[:, h, zlo:QT], start=first, stop=last)
                if last:
                    for h in range(2):
                        den = spool.tile([1, QT], F32, tag="den",
                                         name=f"d{p}_{qt}_{h}")
                        nc.vector.tensor_copy(den, oT[h][HD:HD + 1, :])
                        rec = spool.tile([1, QT], F32, tag="rec",
                                         name=f"r{p}_{qt}_{h}")
                        nc.vector.reciprocal_approx_fast(out=rec, in_=den)
                        bc = spool.tile([HD, QT], F32, tag="bc",
                                        name=f"b{p}_{qt}_{h}")
                        nc.gpsimd.partition_broadcast(bc, rec[0:1, :])
                        nc.vector.tensor_mul(
                            outTn[p][64 * h:64 * h + 64,
                                     qt * QT:(qt + 1) * QT],
                            oT[h][0:HD, :], bc)
                    del oT_live[(p, qt)]
                    if p == 3:
                        # all pairs' rows for this qt are final;
                        # out-projection chains become emittable
                        for tt in range(4 * qt, 4 * qt + 4):
                            for nt in range(2):
                                filler_q.append((4, ("d", tt, nt)))

            all_events = []
            for p in range(NPAIR):
                for qt in range(NQT):
                    kts = [kt for kt in range(NKT) if classes[kt, qt] != 0]
                    for i, kt in enumerate(kts):
                        all_events.append(
                            (p, qt, kt, i == 0, i == len(kts) - 1))

            # ---- ramp: first qk chains, first scores, then v ----------
            # (avs need vext only LAG events in, so exp starts before the
            # v chains instead of behind them)
            with nc.named_scope("ramp"):
                emit_unit(("qk", 0, 0))
                emit_unit(("qk", NPAIR, 0))
                emit_tier(2)
                for ev in all_events[0:LAG]:
                    emit_sexp(ev)
                for kt in range(4):
                    emit_unit(("v", kt))

            # ---- main woven stream ------------------------------------
            with nc.named_scope("attn"):
                # events processed in pairs: both score pairs back-to-back,
                # then both trailing avs — halves the LDWEIGHTS stalls paid
                # entering/leaving the row-packed score matmuls.
                nev = len(all_events)
                for base in range(LAG, nev, 2):
                    evs = all_events[base:base + 2]
                    for ev in evs:
                        p, qt, kt, first, last = ev
                        emit_unit(("qk", p, qt))
                        emit_unit(("qk", NPAIR + p, kt // 4))
                        emit_unit(("v", kt))
                    for ev in evs:
                        emit_sexp(ev)
                    state["quota"] += RATE * len(evs)
                    for idx in range(base, base + len(evs)):
                        j = idx - LAG
                        if j >= 0:
                            emit_av(all_events[j])
                    pop_fillers()
                for j in range(max(0, nev - LAG), nev):
                    emit_av(all_events[j])

            with nc.named_scope("tail"):
                state["quota"] = float("inf")
                pop_fillers()

    nc.compile()
    return nc


def _prepare_inputs(x, mask, w_qkv, b_qkv, w_out):
    classes, bounds = _classify_mask(np.asarray(mask))
    tri_np = (np.arange(QT)[None, :] >= np.arange(KT)[:, None])
    tri_np = tri_np[:, :KT].astype(np.float16)      # [128, 128] triangle
    in_maps = []
    for c in range(NCORES):
        b, g = c // 2, c % 2
        h0 = g * HPC
        # all layouts swizzled to [128 partitions, contiguous payload]
        xT = (x[b].T.astype(np.float16).reshape(NCH, 128, NQT, QT)
              .transpose(1, 2, 0, 3).reshape(128, -1))
        wq = w_qkv[:, h0 * HD:h0 * HD + DC]
        wk = w_qkv[:, D + h0 * HD:D + h0 * HD + DC]
        wv = w_qkv[:, 2 * D + h0 * HD:2 * D + h0 * HD + DC]
        bq = b_qkv[h0 * HD:h0 * HD + DC]
        bk = b_qkv[D + h0 * HD:D + h0 * HD + DC]
        bv = b_qkv[2 * D + h0 * HD:2 * D + h0 * HD + DC]
        wo = w_out[h0 * HD:h0 * HD + DC, :]
        wqk = (np.concatenate([wq, wk], axis=1).astype(np.float16)
               .reshape(NCH, 128, 8, 128).transpose(1, 2, 0, 3)
               .reshape(128, -1))
        in_maps.append({
            "xT": np.ascontiguousarray(xT),
            "wqk": np.ascontiguousarray(wqk),
            "wv": np.ascontiguousarray(
                wv.astype(np.float16).reshape(NCH, 128, DC)
                .transpose(1, 0, 2).reshape(128, -1)),
            "wo": np.ascontiguousarray(
                wo.astype(np.float16).reshape(NPAIR, 128, D)
                .transpose(1, 0, 2).reshape(128, -1)),
            "bqk": np.ascontiguousarray(
                np.concatenate([bq, bk]).reshape(2 * NPAIR, 128).T
                .astype(np.float32)),
            "bv": np.ascontiguousarray(bv[None, :].astype(np.float32)),
            "tri": tri_np,
        })
    return classes, bounds, in_maps


def run(x, mask, w_qkv, b_qkv, w_out, b_out, trace=False):
    classes, bounds, in_maps = _prepare_inputs(
        x, mask, w_qkv, b_qkv, w_out)
    key = (classes.tobytes(), tuple(sorted(bounds.items())))
    if key not in _cache:
        _cache[key] = _build(classes, bounds)
    nc = _cache[key]
    res = bass_utils.run_bass_kernel_spmd(
        nc, in_maps, core_ids=list(range(NCORES)), trace=trace)
    out = np.empty((B, S, D), np.float32)
    bo = np.asarray(b_out, np.float32)
    for b in range(B):
        out[b] = (res.results[2 * b]["outA"].astype(np.float32)
                  + res.results[2 * b + 1]["outA"].astype(np.float32) + bo)
    return out, res


def kernel(x, mask, w_qkv, b_qkv, w_out, b_out):
    out, _ = run(x, mask, w_qkv, b_qkv, w_out, b_out, trace=False)
    return out
